# revision 10
# baseline (speedup 1.0000x reference)
"""BEVFeatureExtractorV2 Trainium2 kernel.

Computes, for each ROI box, 5 sample points (center + 4 edge midpoints of the
rotated box) and bilinearly interpolates a [C,H,W] BEV feature map at those
points, producing [B, N, 5*C].

Sharding: 8 cores = 4 batches x 2 halves of the 512 rois. Each core receives
its batch's feature map (re-laid-out on host) and 256 rois.

Device strategy (per core):
  - Host pre-lays the feature map as table2[y*W+x] = [im[y,x,:], im[y+1,x,:]]
    (shape [H*W, 2C]) so ONE indirect-DMA descriptor (4KB) fetches all 4
    bilinear neighbors of a point: entries e and e+1 give pixels
    (y0,x0),(y0+1,x0),(y0,x0+1),(y0+1,x0+1).
  - On device: compute the 5 points per roi (sin on ACT engine; floor via
    f32 mod), int32 gather indices and the 4 bilinear weights; then for each
    of 10 tiles of 128 points: indirect-gather [128, 4C], multiply by the
    per-point weight vector broadcast over channels, fold 4 chunks with 2
    adds, store [128, C] to the output slab.
"""

import os
import numpy as np

import concourse.bass as bass
import concourse.bacc as bacc
import concourse.tile as tile
from concourse import mybir
from concourse.bass_utils import run_bass_kernel_spmd

F32 = mybir.dt.float32
I32 = mybir.dt.int32

B, N, C, H, W = 4, 512, 256, 256, 256
NCORES = 8
NR = N * B // NCORES          # rois per core = 256
NPT = 5                       # sample points per roi
P = 128                       # partitions
NT = NR // P                  # roi tiles per core = 2
PC_START = -51.2
INV_VOX = 2.5                 # 1 / (0.1 * 4)
BIAS = -PC_START * INV_VOX    # 128.0

_CACHED = {}


def build_program(repeat=1, loop_iters=None, groups=(5,), bufs=2):
    import contextlib
    nc = bacc.Bacc("TRN2", target_bir_lowering=False, debug=False,
                   enable_asserts=False)
    table = nc.dram_tensor("table2", [H * W, 2 * C], F32, kind="ExternalInput").ap()
    rois = nc.dram_tensor("rois", [NR, 7], F32, kind="ExternalInput").ap()
    out = nc.dram_tensor("out", [NR, NPT * C], F32, kind="ExternalOutput").ap()

    NJ = NT * NPT  # 10 gather tiles
    with tile.TileContext(nc) as tc:
        with tc.tile_pool(name="coord", bufs=min(2, repeat)) as cp, \
             tc.tile_pool(name="gather", bufs=bufs) as gp, \
             tc.tile_pool(name="mul", bufs=bufs) as mp, \
             tc.tile_pool(name="fold", bufs=bufs) as sp, \
             tc.tile_pool(name="outp", bufs=bufs) as op, \
             (tc.For_i(0, loop_iters, 1) if loop_iters
              else contextlib.nullcontext()):
          for _rep in range(repeat):
            # ---- load rois: [256,7] -> [128, (t d)] -------------------
            R = cp.tile([P, NT * 7], F32)
            R3 = R[:].rearrange("p (t d) -> p t d", t=NT)
            nc.gpsimd.dma_start(R3, rois.rearrange("(t p) d -> p t d", t=NT))

            cx = R3[:, :, 0]
            cy = R3[:, :, 1]
            ry = R3[:, :, 6]

            # ---- trig ------------------------------------------------
            zero = cp.tile([P, 1], F32)
            halfpi = cp.tile([P, 1], F32)
            nc.vector.memset(zero[:], 0.0)
            nc.vector.memset(halfpi[:], float(np.pi / 2))
            trig = cp.tile([P, 2 * NT], F32)
            t3 = trig[:].rearrange("p (a t) -> p a t", a=2)
            sn = t3[:, 0, :]   # sin(ry)
            cs = t3[:, 1, :]   # cos(ry)
            nc.scalar.activation(sn, ry, mybir.ActivationFunctionType.Sin,
                                 bias=zero[:])
            # cos(x) = sin(pi/2 - |x|), argument stays within [-pi/2, pi/2]
            ab = cp.tile([P, NT], F32)
            nc.scalar.activation(ab[:], ry, mybir.ActivationFunctionType.Abs,
                                 bias=zero[:])
            nc.scalar.activation(cs, ab[:], mybir.ActivationFunctionType.Sin,
                                 bias=halfpi[:], scale=-1.0)

            # ---- half dims ------------------------------------------
            hd = cp.tile([P, 2 * NT], F32)
            h3 = hd[:].rearrange("p (a t) -> p a t", a=2)
            hx = h3[:, 0, :]
            hy = h3[:, 1, :]
            nc.vector.tensor_scalar_mul(hx, R3[:, :, 3], 0.5)
            nc.vector.tensor_scalar_mul(hy, R3[:, :, 4], 0.5)

            # ---- rotated offsets ------------------------------------
            rot = cp.tile([P, 4 * NT], F32)
            r3 = rot[:].rearrange("p (a t) -> p a t", a=4)
            rxc, rxs, rys, ryc = (r3[:, a, :] for a in range(4))
            nc.vector.tensor_mul(rxc, hx, cs)
            nc.vector.tensor_mul(rxs, hx, sn)
            nc.vector.tensor_mul(rys, hy, sn)
            nc.vector.tensor_mul(ryc, hy, cs)

            # ---- 5 points per roi: [128, t, k] ----------------------
            px = cp.tile([P, NJ], F32)
            py = cp.tile([P, NJ], F32)
            px3 = px[:].rearrange("p (t k) -> p t k", t=NT)
            py3 = py[:].rearrange("p (t k) -> p t k", t=NT)
            nc.vector.tensor_copy(px3[:, :, 0], cx)
            nc.vector.tensor_copy(py3[:, :, 0], cy)
            nc.vector.tensor_sub(px3[:, :, 1], cx, rxc)   # front
            nc.vector.tensor_add(py3[:, :, 1], cy, rxs)
            nc.vector.tensor_add(px3[:, :, 2], cx, rxc)   # back
            nc.vector.tensor_sub(py3[:, :, 2], cy, rxs)
            nc.vector.tensor_sub(px3[:, :, 3], cx, rys)   # left
            nc.vector.tensor_sub(py3[:, :, 3], cy, ryc)
            nc.vector.tensor_add(px3[:, :, 4], cx, rys)   # right
            nc.vector.tensor_add(py3[:, :, 4], cy, ryc)

            # ---- pixel coords, floor, weights -----------------------
            xs = cp.tile([P, NJ], F32)
            ys = cp.tile([P, NJ], F32)
            nc.scalar.activation(xs[:], px[:], mybir.ActivationFunctionType.Copy,
                                 bias=BIAS, scale=INV_VOX)
            nc.scalar.activation(ys[:], py[:], mybir.ActivationFunctionType.Copy,
                                 bias=BIAS, scale=INV_VOX)

            # floor(): int32 roundtrip (rounding-mode-agnostic) + is_lt fixup
            xf = cp.tile([P, NJ], F32)
            yf = cp.tile([P, NJ], F32)
            fx = cp.tile([P, NJ], F32)
            fy = cp.tile([P, NJ], F32)
            xi = cp.tile([P, NJ], I32)
            yi = cp.tile([P, NJ], I32)
            corr = cp.tile([P, NJ], F32)
            nc.vector.tensor_copy(xi[:], xs[:])
            nc.vector.tensor_copy(xf[:], xi[:])
            nc.vector.tensor_sub(fx[:], xs[:], xf[:])        # in (-1, 1)
            nc.vector.tensor_scalar(corr[:], fx[:], 0.0, None,
                                    mybir.AluOpType.is_lt)   # 1.0 where xf > xs
            nc.vector.tensor_sub(xf[:], xf[:], corr[:])
            nc.vector.tensor_sub(fx[:], xs[:], xf[:])
            nc.vector.tensor_copy(yi[:], ys[:])
            nc.vector.tensor_copy(yf[:], yi[:])
            nc.vector.tensor_sub(fy[:], ys[:], yf[:])
            nc.vector.tensor_scalar(corr[:], fy[:], 0.0, None,
                                    mybir.AluOpType.is_lt)
            nc.vector.tensor_sub(yf[:], yf[:], corr[:])
            nc.vector.tensor_sub(fy[:], ys[:], yf[:])

            gx = cp.tile([P, NJ], F32)
            gy = cp.tile([P, NJ], F32)
            nc.vector.tensor_scalar(gx[:], fx[:], -1.0, 1.0,
                                    mybir.AluOpType.mult, mybir.AluOpType.add)
            nc.vector.tensor_scalar(gy[:], fy[:], -1.0, 1.0,
                                    mybir.AluOpType.mult, mybir.AluOpType.add)

            # weights, interleaved [wa wb wc wd] per point
            Wt = cp.tile([P, 4 * NJ], F32)
            W3 = Wt[:].rearrange("p (j w) -> p j w", w=4)
            nc.vector.tensor_mul(W3[:, :, 0], gx[:], gy[:])
            nc.vector.tensor_mul(W3[:, :, 1], gx[:], fy[:])
            nc.vector.tensor_mul(W3[:, :, 2], fx[:], gy[:])
            nc.vector.tensor_mul(W3[:, :, 3], fx[:], fy[:])

            # gather index = yf*W + xf  (exact in f32, then exact int cast)
            idxf = cp.tile([P, NJ], F32)
            nc.vector.tensor_scalar_mul(idxf[:], yf[:], float(W))
            nc.vector.tensor_add(idxf[:], idxf[:], xf[:])
            idx = cp.tile([P, NJ], I32)
            nc.vector.tensor_copy(idx[:], idxf[:])

            # ---- gather + weighted fold, grouped --------------------
            # groups: list of k-group sizes per roi-half, e.g. [5] or [3,2]
            for t in range(NT):
                k0 = 0
                for g in groups:
                    j0 = t * NPT + k0
                    G = gp.tile([P, g * 4 * C], F32, tag="G")
                    nc.gpsimd.indirect_dma_start(
                        out=G[:].rearrange("p (j c) -> p j c", j=g),
                        out_offset=None,
                        in_=table,
                        in_offset=bass.IndirectOffsetOnAxis(
                            ap=idx[:, j0:j0 + g], axis=0),
                    )
                    M = mp.tile([P, g * 4 * C], F32, tag="M")
                    G4 = G[:].rearrange("p (j a c) -> p j a c", j=g, a=4)
                    M4 = M[:].rearrange("p (j a c) -> p j a c", j=g, a=4)
                    nc.vector.tensor_mul(
                        M4, G4,
                        W3[:, j0:j0 + g, :].unsqueeze(3).to_broadcast(
                            [P, g, 4, C]),
                    )
                    S = sp.tile([P, g * 2 * C], F32, tag="S")
                    S4 = S[:].rearrange("p (j a c) -> p j a c", j=g, a=2)
                    nc.vector.tensor_add(S4, M4[:, :, 0:2, :], M4[:, :, 2:4, :])
                    O = op.tile([P, g * C], F32, tag="O")
                    O3 = O[:].rearrange("p (j c) -> p j c", j=g)
                    nc.vector.tensor_add(O3, S4[:, :, 0, :], S4[:, :, 1, :])
                    nc.sync.dma_start(
                        out[t * P:(t + 1) * P, k0 * C:(k0 + g) * C], O3)
                    k0 += g
    nc.compile()
    return nc


def _get_program():
    if "nc" not in _CACHED:
        _CACHED["nc"] = build_program()
    return _CACHED["nc"]


def _make_table2(feats):
    """feats: [B,C,H,W] f32 -> list of B arrays [H*W, 2C] (channel-last,
    row y and y+1 concatenated)."""
    tables = []
    for b in range(B):
        bev = np.ascontiguousarray(feats[b].transpose(1, 2, 0))  # [H,W,C]
        nxt = bev[np.minimum(np.arange(H) + 1, H - 1)]           # [H,W,C]
        t2 = np.concatenate([bev, nxt], axis=2)                  # [H,W,2C]
        tables.append(np.ascontiguousarray(t2.reshape(H * W, 2 * C)))
    return tables


def kernel(spatial_features_2d, rois, _want_results=False):
    feats = np.asarray(spatial_features_2d, dtype=np.float32)
    rois_np = np.asarray(rois, dtype=np.float32)
    assert feats.shape == (B, C, H, W) and rois_np.shape == (B, N, 7)

    nc = _get_program()
    tables = _make_table2(feats)
    in_maps = []
    for core in range(NCORES):
        b, h = divmod(core, 2)
        in_maps.append({
            "table2": tables[b],
            "rois": np.ascontiguousarray(rois_np[b, h * NR:(h + 1) * NR]),
        })

    res = run_bass_kernel_spmd(
        nc, in_maps, list(range(NCORES)),
        trace=bool(int(os.environ.get("BEV_TRACE", "0"))),
    )

    out = np.empty((B, N, NPT * C), dtype=np.float32)
    for core in range(NCORES):
        b, h = divmod(core, 2)
        out[b, h * NR:(h + 1) * NR] = res.results[core]["out"]
    if _want_results:
        return out, res
    return out


# revision 13
# speedup vs baseline: 42.5123x; 42.5123x over previous
"""BEVFeatureExtractorV2 Trainium2 kernel.

Computes, for each ROI box, 5 sample points (center + 4 edge midpoints of the
rotated box) and bilinearly interpolates a [C,H,W] BEV feature map at those
points, producing [B, N, 5*C].

Sharding: 8 cores = 4 batches x 2 halves of the 512 rois. Each core receives
its batch's feature map (re-laid-out on host) and 256 rois.

Device strategy (per core):
  - Host pre-lays the feature map as table2[y*W+x] = [im[y,x,:], im[y+1,x,:]]
    (shape [H*W, 2C]) so ONE indirect-DMA descriptor (4KB) fetches all 4
    bilinear neighbors of a point: entries e and e+1 give pixels
    (y0,x0),(y0+1,x0),(y0,x0+1),(y0+1,x0+1).
  - On device: compute the 5 points per roi (sin on ACT engine; floor via
    f32 mod), int32 gather indices and the 4 bilinear weights; then for each
    of 10 tiles of 128 points: indirect-gather [128, 4C], multiply by the
    per-point weight vector broadcast over channels, fold 4 chunks with 2
    adds, store [128, C] to the output slab.
"""

import os
import numpy as np

import concourse.bass as bass
import concourse.bacc as bacc
import concourse.tile as tile
from concourse import mybir
from concourse.bass_utils import run_bass_kernel_spmd

F32 = mybir.dt.float32
I32 = mybir.dt.int32

B, N, C, H, W = 4, 512, 256, 256, 256
NCORES = 8
NR = N * B // NCORES          # rois per core = 256
NPT = 5                       # sample points per roi
P = 128                       # partitions
NT = NR // P                  # roi tiles per core = 2
PC_START = -51.2
INV_VOX = 2.5                 # 1 / (0.1 * 4)
BIAS = -PC_START * INV_VOX    # 128.0

_CACHED = {}


def build_program(repeat=1, loop_iters=None, bufs=(4, 3, 3, 4)):
    import contextlib
    nc = bacc.Bacc("TRN2", target_bir_lowering=False, debug=False,
                   enable_asserts=False)
    table = nc.dram_tensor("table2", [H * W, 2 * C], F32, kind="ExternalInput").ap()
    rois = nc.dram_tensor("rois", [NR, 7], F32, kind="ExternalInput").ap()
    out = nc.dram_tensor("out", [NR, NPT * C], F32, kind="ExternalOutput").ap()

    NJ = NT * NPT  # 10 gather tiles
    with tile.TileContext(nc) as tc:
        with tc.tile_pool(name="coord", bufs=min(2, repeat)) as cp, \
             tc.tile_pool(name="gather", bufs=bufs[0]) as gp, \
             tc.tile_pool(name="mul", bufs=bufs[1]) as mp, \
             tc.tile_pool(name="fold", bufs=bufs[2]) as sp, \
             tc.tile_pool(name="outp", bufs=bufs[3]) as op, \
             (tc.For_i(0, loop_iters, 1) if loop_iters
              else contextlib.nullcontext()):
          for _rep in range(repeat):
            # ---- load rois: [256,7] -> [128, (t d)] -------------------
            R = cp.tile([P, NT * 7], F32)
            R3 = R[:].rearrange("p (t d) -> p t d", t=NT)
            nc.gpsimd.dma_start(R3, rois.rearrange("(t p) d -> p t d", t=NT))

            cx = R3[:, :, 0]
            cy = R3[:, :, 1]
            ry = R3[:, :, 6]

            # ---- trig ------------------------------------------------
            zero = cp.tile([P, 1], F32)
            halfpi = cp.tile([P, 1], F32)
            nc.vector.memset(zero[:], 0.0)
            nc.vector.memset(halfpi[:], float(np.pi / 2))
            trig = cp.tile([P, 2 * NT], F32)
            t3 = trig[:].rearrange("p (a t) -> p a t", a=2)
            sn = t3[:, 0, :]   # sin(ry)
            cs = t3[:, 1, :]   # cos(ry)
            nc.scalar.activation(sn, ry, mybir.ActivationFunctionType.Sin,
                                 bias=zero[:])
            # cos(x) = sin(pi/2 - |x|), argument stays within [-pi/2, pi/2]
            ab = cp.tile([P, NT], F32)
            nc.scalar.activation(ab[:], ry, mybir.ActivationFunctionType.Abs,
                                 bias=zero[:])
            nc.scalar.activation(cs, ab[:], mybir.ActivationFunctionType.Sin,
                                 bias=halfpi[:], scale=-1.0)

            # ---- half dims ------------------------------------------
            hd = cp.tile([P, 2 * NT], F32)
            h3 = hd[:].rearrange("p (a t) -> p a t", a=2)
            hx = h3[:, 0, :]
            hy = h3[:, 1, :]
            nc.vector.tensor_scalar_mul(hx, R3[:, :, 3], 0.5)
            nc.vector.tensor_scalar_mul(hy, R3[:, :, 4], 0.5)

            # ---- rotated offsets ------------------------------------
            rot = cp.tile([P, 4 * NT], F32)
            r3 = rot[:].rearrange("p (a t) -> p a t", a=4)
            rxc, rxs, rys, ryc = (r3[:, a, :] for a in range(4))
            nc.vector.tensor_mul(rxc, hx, cs)
            nc.vector.tensor_mul(rxs, hx, sn)
            nc.vector.tensor_mul(rys, hy, sn)
            nc.vector.tensor_mul(ryc, hy, cs)

            # ---- 5 points per roi: [128, t, k] ----------------------
            px = cp.tile([P, NJ], F32)
            py = cp.tile([P, NJ], F32)
            px3 = px[:].rearrange("p (t k) -> p t k", t=NT)
            py3 = py[:].rearrange("p (t k) -> p t k", t=NT)
            nc.vector.tensor_copy(px3[:, :, 0], cx)
            nc.vector.tensor_copy(py3[:, :, 0], cy)
            nc.vector.tensor_sub(px3[:, :, 1], cx, rxc)   # front
            nc.vector.tensor_add(py3[:, :, 1], cy, rxs)
            nc.vector.tensor_add(px3[:, :, 2], cx, rxc)   # back
            nc.vector.tensor_sub(py3[:, :, 2], cy, rxs)
            nc.vector.tensor_sub(px3[:, :, 3], cx, rys)   # left
            nc.vector.tensor_sub(py3[:, :, 3], cy, ryc)
            nc.vector.tensor_add(px3[:, :, 4], cx, rys)   # right
            nc.vector.tensor_add(py3[:, :, 4], cy, ryc)

            # ---- pixel coords, floor, weights -----------------------
            xs = cp.tile([P, NJ], F32)
            ys = cp.tile([P, NJ], F32)
            nc.scalar.activation(xs[:], px[:], mybir.ActivationFunctionType.Copy,
                                 bias=BIAS, scale=INV_VOX)
            nc.scalar.activation(ys[:], py[:], mybir.ActivationFunctionType.Copy,
                                 bias=BIAS, scale=INV_VOX)

            # floor(): int32 roundtrip (rounding-mode-agnostic) + is_lt fixup
            xf = cp.tile([P, NJ], F32)
            yf = cp.tile([P, NJ], F32)
            fx = cp.tile([P, NJ], F32)
            fy = cp.tile([P, NJ], F32)
            xi = cp.tile([P, NJ], I32)
            yi = cp.tile([P, NJ], I32)
            corr = cp.tile([P, NJ], F32)
            nc.vector.tensor_copy(xi[:], xs[:])
            nc.vector.tensor_copy(xf[:], xi[:])
            nc.vector.tensor_sub(fx[:], xs[:], xf[:])        # in (-1, 1)
            nc.vector.tensor_scalar(corr[:], fx[:], 0.0, None,
                                    mybir.AluOpType.is_lt)   # 1.0 where xf > xs
            nc.vector.tensor_sub(xf[:], xf[:], corr[:])
            nc.vector.tensor_sub(fx[:], xs[:], xf[:])
            nc.vector.tensor_copy(yi[:], ys[:])
            nc.vector.tensor_copy(yf[:], yi[:])
            nc.vector.tensor_sub(fy[:], ys[:], yf[:])
            nc.vector.tensor_scalar(corr[:], fy[:], 0.0, None,
                                    mybir.AluOpType.is_lt)
            nc.vector.tensor_sub(yf[:], yf[:], corr[:])
            nc.vector.tensor_sub(fy[:], ys[:], yf[:])

            gx = cp.tile([P, NJ], F32)
            gy = cp.tile([P, NJ], F32)
            nc.vector.tensor_scalar(gx[:], fx[:], -1.0, 1.0,
                                    mybir.AluOpType.mult, mybir.AluOpType.add)
            nc.vector.tensor_scalar(gy[:], fy[:], -1.0, 1.0,
                                    mybir.AluOpType.mult, mybir.AluOpType.add)

            # weights, interleaved [wa wb wc wd] per point
            Wt = cp.tile([P, 4 * NJ], F32)
            W3 = Wt[:].rearrange("p (j w) -> p j w", w=4)
            nc.vector.tensor_mul(W3[:, :, 0], gx[:], gy[:])
            nc.vector.tensor_mul(W3[:, :, 1], gx[:], fy[:])
            nc.vector.tensor_mul(W3[:, :, 2], fx[:], gy[:])
            nc.vector.tensor_mul(W3[:, :, 3], fx[:], fy[:])

            # gather index = yf*W + xf  (exact in f32, then exact int cast)
            idxf = cp.tile([P, NJ], F32)
            nc.vector.tensor_scalar_mul(idxf[:], yf[:], float(W))
            nc.vector.tensor_add(idxf[:], idxf[:], xf[:])
            idx = cp.tile([P, NJ], I32)
            nc.vector.tensor_copy(idx[:], idxf[:])

            # ---- gather + weighted fold per (t, k) tile -------------
            for t in range(NT):
                for k in range(NPT):
                    j = t * NPT + k
                    G = gp.tile([P, 4 * C], F32, tag="G")
                    nc.gpsimd.indirect_dma_start(
                        out=G[:],
                        out_offset=None,
                        in_=table,
                        in_offset=bass.IndirectOffsetOnAxis(
                            ap=idx[:, j:j + 1], axis=0),
                    )
                    # chunks 0,1 weighted on DVE; chunks 2,3 on ACT (idle)
                    M = mp.tile([P, 4 * C], F32, tag="M")
                    nc.vector.tensor_mul(
                        M[:, :2 * C].rearrange("p (a c) -> p a c", a=2),
                        G[:, :2 * C].rearrange("p (a c) -> p a c", a=2),
                        W3[:, j, 0:2].unsqueeze(2).to_broadcast([P, 2, C]),
                    )
                    nc.scalar.activation(
                        M[:, 2 * C:3 * C], G[:, 2 * C:3 * C],
                        mybir.ActivationFunctionType.Copy,
                        bias=0.0, scale=W3[:, j, 2:3])
                    nc.scalar.activation(
                        M[:, 3 * C:4 * C], G[:, 3 * C:4 * C],
                        mybir.ActivationFunctionType.Copy,
                        bias=0.0, scale=W3[:, j, 3:4])
                    S = sp.tile([P, 2 * C], F32, tag="S")
                    nc.vector.tensor_add(S[:], M[:, :2 * C], M[:, 2 * C:])
                    O = op.tile([P, C], F32, tag="O")
                    nc.vector.tensor_add(O[:], S[:, :C], S[:, C:])
                    nc.sync.dma_start(
                        out[t * P:(t + 1) * P, k * C:(k + 1) * C], O[:])
    nc.compile()
    return nc


def _get_program():
    if "nc" not in _CACHED:
        _CACHED["nc"] = build_program()
    return _CACHED["nc"]


def _make_table2(feats):
    """feats: [B,C,H,W] f32 -> list of B arrays [H*W, 2C] (channel-last,
    row y and y+1 concatenated)."""
    tables = []
    for b in range(B):
        bev = np.ascontiguousarray(feats[b].transpose(1, 2, 0))  # [H,W,C]
        nxt = bev[np.minimum(np.arange(H) + 1, H - 1)]           # [H,W,C]
        t2 = np.concatenate([bev, nxt], axis=2)                  # [H,W,2C]
        tables.append(np.ascontiguousarray(t2.reshape(H * W, 2 * C)))
    return tables


def kernel(spatial_features_2d, rois, _want_results=False):
    feats = np.asarray(spatial_features_2d, dtype=np.float32)
    rois_np = np.asarray(rois, dtype=np.float32)
    assert feats.shape == (B, C, H, W) and rois_np.shape == (B, N, 7)

    nc = _get_program()
    tables = _make_table2(feats)
    in_maps = []
    for core in range(NCORES):
        b, h = divmod(core, 2)
        in_maps.append({
            "table2": tables[b],
            "rois": np.ascontiguousarray(rois_np[b, h * NR:(h + 1) * NR]),
        })

    res = run_bass_kernel_spmd(
        nc, in_maps, list(range(NCORES)),
        trace=bool(int(os.environ.get("BEV_TRACE", "0"))),
    )

    out = np.empty((B, N, NPT * C), dtype=np.float32)
    for core in range(NCORES):
        b, h = divmod(core, 2)
        out[b, h * NR:(h + 1) * NR] = res.results[core]["out"]
    if _want_results:
        return out, res
    return out
